# revision 2
# baseline (speedup 1.0000x reference)
"""Trainium2 Bass kernel for nn_BiLinearInteractionLayer.

Math: x:(B=4096, F=32, D=64) f32, W:(P=496, D=64, D=64) f32 (torch Linear
layout: out_e = sum_d in_d * W[e, d]).  For each pair p=(i,j), i<j:
    out[b, p, e] = (sum_d x[b,i,d] * W[p,e,d]) * x[b,j,e]

Strategy (data-parallel over batch, 8 cores x 512 rows):

The harness gate is rel_err < 2e-2 (normalized by the global max), so the
whole pipeline runs in fp16: single-pass k=64 fp16 matmuls (fp32 PSUM
accumulate), fp16 elementwise multiply, fp16 output stores that the host
widens back to f32.  Measured numerics: ~6e-4 rel err, 30x under the gate.
Versus the fp32-exact baseline this halves PE matmul passes, removes the
hi/lo split entirely, and halves HBM store traffic (the dominant cost:
output is 65MB/core in f32, 32.5MB in fp16).

Host preformatting: x is shipped twice in fp16 - natively (BL, F*D) for
the elementwise right-field operand, and pre-transposed per batch-tile
(64, bt*F*128) for the matmul stationary operand - plus W transposed to
wt[d, p*64+e] in fp16.  No on-chip transposes: the PE runs nothing but
the pair matmuls.

Per 128-row batch tile, per left field i, pairs in chunks of <=8
(cn*64 <= 512 = one PSUM bank): matmul -> PSUM f32, then the elementwise
multiply with the natively-laid-out right fields is routed round-robin
across three paths to balance engine load:
  D: DVE tensor_mul direct from PSUM (1x mode, fp32 operand)
  A: ACT evicts PSUM->SBUF fp16, DVE scalar_tensor_tensor all-fp16/SBUF
     (eligible for the 4x_2p DVE perf mode)
  P: ACT evicts, GPSIMD does the multiply (GPSIMD has no PSUM port)
Outputs accumulate in per-group (4 left fields) fp16 tiles and store once
per group: 8 stores/bt with 4-15KB contiguous runs per partition.

HBM traffic/core: 32.5MB out + 4MB wt + 4MB x (2 layouts) = 40.5MB.
"""
import numpy as np

import concourse.bacc as bacc
import concourse.tile as tile
import concourse.mybir as mybir
from concourse.bass_utils import run_bass_kernel_spmd

B = 4096
F = 32
D = 64
P = F * (F - 1) // 2  # 496
N_CORES = 8
BL = B // N_CORES     # 512 rows per core
BT = 128              # batch tile (SBUF partitions)
NBT = BL // BT        # 4 batch tiles per core
CHUNK = 8             # pairs per matmul chunk (8*64 = 512 = one PSUM bank)
TGROUP = 4            # left fields per output-store group
NLEFT = F - 1         # left fields 0..30

f32 = mybir.dt.float32
f16 = mybir.dt.float16

# Combine-route mix: A = ACT evict + DVE fp16 mul, D = DVE direct from
# PSUM, P = ACT evict + GPSIMD mul.  Fractions ~(0.53, 0.29, 0.18)
# balance DVE / ACT / Pool busy time at ~90us each, under the ~115us DMA
# floor.  Spread (not bursty) so in-order engine queues don't stall.
_ROUTE_PATTERN = ("A", "D", "A", "P", "A", "D", "A", "A", "D", "P",
                  "A", "D", "A", "A", "D", "A", "P")

_nc_cache = None


def _off(i):
    """Pair index of the first pair with left field i."""
    return 31 * i - i * (i - 1) // 2


def _chunks(npair):
    out = []
    c0 = 0
    rem = npair
    while rem > 0:
        if rem > CHUNK:
            take = CHUNK if rem - CHUNK >= 4 or rem - CHUNK == 0 else rem - 4
        else:
            take = rem
        out.append((c0, take))
        c0 += take
        rem -= take
    return out


_GROUPS = [(g0, min(TGROUP, NLEFT - g0)) for g0 in range(0, NLEFT, TGROUP)]


def _build():
    nc = bacc.Bacc("TRN2", target_bir_lowering=False, debug=False,
                   num_devices=N_CORES)
    x_in = nc.dram_tensor("x", [BL, F * D], f16, kind="ExternalInput").ap()
    # xt[d, ((bt*F + f)*BT + r)] = x[bt*BT + r, f, d]
    xt_in = nc.dram_tensor("xt", [D, NBT * F * BT], f16,
                           kind="ExternalInput").ap()
    # wt[d, p*D + e] = W[p, e, d]
    wt_in = nc.dram_tensor("wt", [D, P * D], f16, kind="ExternalInput").ap()
    out = nc.dram_tensor("out", [BL, P * D], f16, kind="ExternalOutput").ap()

    mult = mybir.AluOpType.mult

    with tile.TileContext(nc) as tc:
        with (
            tc.tile_pool(name="consts", bufs=1) as consts,
            tc.tile_pool(name="xp", bufs=2) as xp,
            tc.tile_pool(name="xtp", bufs=2) as xtp,
            tc.tile_pool(name="otp", bufs=3) as otp,
            tc.tile_pool(name="pm16p", bufs=6) as pm16p,
            tc.tile_pool(name="psm", bufs=8, space="PSUM") as psm,
        ):
            # one weight tile per field group -> matmuls of group g only
            # depend on weight load g
            wt_g = []
            for gi, (g0, gn) in enumerate(_GROUPS):
                c0 = _off(g0) * D
                c1 = _off(g0 + gn) * D
                wt_g.append(consts.tile([D, c1 - c0], f16, tag=f"wt{gi}"))

            route_idx = 0
            for bt in range(NBT):
                rows = slice(bt * BT, (bt + 1) * BT)
                x_tile = xp.tile([BT, F * D], f16, tag="x")
                nc.sync.dma_start(out=x_tile, in_=x_in[rows, :])
                xt_tile = xtp.tile([D, F * BT], f16, tag="xt")
                nc.sync.dma_start(
                    out=xt_tile, in_=xt_in[:, bt * F * BT:(bt + 1) * F * BT])
                if bt == 0:
                    for gi, (g0, gn) in enumerate(_GROUPS):
                        c0 = _off(g0) * D
                        c1 = _off(g0 + gn) * D
                        nc.sync.dma_start(out=wt_g[gi], in_=wt_in[:, c0:c1])

                for gi, (g0, gn) in enumerate(_GROUPS):
                    gbase = _off(g0) * D
                    gsz = (_off(g0 + gn) - _off(g0)) * D
                    ot = otp.tile([BT, gsz], f16, tag="ot")
                    for i in range(g0, g0 + gn):
                        npair = F - 1 - i  # pairs (i, i+1..31), consecutive
                        p0 = _off(i)
                        lhsT = xt_tile[:, i * BT:(i + 1) * BT]  # [64, 128]
                        for c0, cn in _chunks(npair):
                            n = cn * D
                            ws = (p0 + c0) * D - gbase
                            pm = psm.tile([BT, n], f32, tag="mm")
                            nc.tensor.matmul(pm, lhsT,
                                             wt_g[gi][:, ws:ws + n],
                                             start=True, stop=True)
                            xj = x_tile[:, (i + 1 + c0) * D:
                                        (i + 1 + c0 + cn) * D]
                            ot_sl = ot[:, ws:ws + n]
                            route = _ROUTE_PATTERN[route_idx
                                                   % len(_ROUTE_PATTERN)]
                            route_idx += 1
                            if route == "D":
                                nc.vector.tensor_mul(ot_sl, pm, xj)
                            else:
                                pm16 = pm16p.tile([BT, n], f16, tag="pm16")
                                nc.scalar.copy(pm16, pm)
                                if route == "A":
                                    nc.vector.scalar_tensor_tensor(
                                        ot_sl, pm16, 1.0, xj, mult, mult)
                                else:
                                    nc.gpsimd.tensor_mul(ot_sl, pm16, xj)
                    nc.sync.dma_start(out=out[rows, gbase:gbase + gsz],
                                      in_=ot)
    nc.compile()
    return nc


def _get_nc():
    global _nc_cache
    if _nc_cache is None:
        _nc_cache = _build()
    return _nc_cache


def _prep_inputs(x, W):
    x16 = np.asarray(x, dtype=np.float16).reshape(N_CORES, BL, F * D)
    # per-core pre-transposed layout: [D, NBT, F, BT] flattened
    xt = np.ascontiguousarray(
        x16.reshape(N_CORES, NBT, BT, F, D).transpose(0, 4, 1, 3, 2)
    ).reshape(N_CORES, D, NBT * F * BT)
    wt = np.ascontiguousarray(
        np.asarray(W, dtype=np.float32).transpose(2, 0, 1)
    ).reshape(D, P * D).astype(np.float16)
    x16 = np.ascontiguousarray(x16)
    return x16, xt, wt


def _run(x, W, trace=False, trace_kwargs=None):
    x16, xt, wt = _prep_inputs(x, W)
    in_maps = [{"x": x16[c], "xt": xt[c], "wt": wt}
               for c in range(N_CORES)]
    res = run_bass_kernel_spmd(_get_nc(), in_maps, list(range(N_CORES)),
                               trace=trace, **(trace_kwargs or {}))
    outs = [res.results[c]["out"].reshape(BL, P, D) for c in range(N_CORES)]
    return np.concatenate(outs, axis=0).astype(np.float32), res


def kernel(x, W):
    out, _ = _run(x, W)
    return out


# revision 3
# speedup vs baseline: 1.3446x; 1.3446x over previous
"""Trainium2 Bass kernel for nn_BiLinearInteractionLayer.

Math: x:(B=4096, F=32, D=64) f32, W:(P=496, D=64, D=64) f32 (torch Linear
layout: out_e = sum_d in_d * W[e, d]).  For each pair p=(i,j), i<j:
    out[b, p, e] = (sum_d x[b,i,d] * W[p,e,d]) * x[b,j,e]

Strategy (data-parallel over batch, 8 cores x 512 rows):

The harness gate is rel_err < 2e-2 (normalized by the global max), so the
whole pipeline runs in fp16: single-pass k=64 fp16 matmuls (fp32 PSUM
accumulate), fp16 elementwise multiply, fp16 output stores that the host
widens back to f32.  Measured numerics: ~6e-4 rel err, 30x under the gate.
Versus the fp32-exact baseline this halves PE matmul passes, removes the
hi/lo split entirely, and halves HBM store traffic (the dominant cost:
output is 65MB/core in f32, 32.5MB in fp16).

Host preformatting: x is shipped twice in fp16 - natively (BL, F*D) for
the elementwise right-field operand, and pre-transposed per batch-tile
(64, bt*F*128) for the matmul stationary operand - plus W transposed to
wt[d, p*64+e] in fp16.  No on-chip transposes: the PE runs nothing but
the pair matmuls.

Per 128-row batch tile, per left field i, pairs in chunks of <=8
(cn*64 <= 512 = one PSUM bank): matmul -> PSUM f32, then the elementwise
multiply with the natively-laid-out right fields is routed round-robin
across three paths to balance engine load:
  D: DVE tensor_mul direct from PSUM (1x mode, fp32 operand)
  A: ACT evicts PSUM->SBUF fp16, DVE scalar_tensor_tensor all-fp16/SBUF
     (eligible for the 4x_2p DVE perf mode)
  P: ACT evicts, GPSIMD does the multiply (GPSIMD has no PSUM port)
Outputs accumulate in per-group (4 left fields) fp16 tiles and store once
per group: 8 stores/bt with 4-15KB contiguous runs per partition.

HBM traffic/core: 32.5MB out + 4MB wt + 4MB x (2 layouts) = 40.5MB.
"""
import numpy as np

import concourse.bacc as bacc
import concourse.tile as tile
import concourse.mybir as mybir
from concourse.bass_utils import run_bass_kernel_spmd

B = 4096
F = 32
D = 64
P = F * (F - 1) // 2  # 496
N_CORES = 8
BL = B // N_CORES     # 512 rows per core
BT = 128              # batch tile (SBUF partitions)
NBT = BL // BT        # 4 batch tiles per core
CHUNK = 8             # pairs per matmul chunk (8*64 = 512 = one PSUM bank)
TGROUP = 4            # left fields per output-store group
NLEFT = F - 1         # left fields 0..30

f32 = mybir.dt.float32
f16 = mybir.dt.float16

# Combine-route mix: A = ACT evict + DVE fp16 mul, D = DVE direct from
# PSUM, P = ACT evict + GPSIMD mul.  Fractions ~(0.53, 0.29, 0.18)
# balance DVE / ACT / Pool busy time at ~90us each, under the ~115us DMA
# floor.  Spread (not bursty) so in-order engine queues don't stall.
_ROUTE_PATTERN = ("A", "D", "A", "P", "A", "D", "A", "A", "D", "P",
                  "A", "D", "A", "A", "D", "A", "P")

_nc_cache = None


def _off(i):
    """Pair index of the first pair with left field i."""
    return 31 * i - i * (i - 1) // 2


def _chunks(npair):
    out = []
    c0 = 0
    rem = npair
    while rem > 0:
        if rem > CHUNK:
            take = CHUNK if rem - CHUNK >= 4 or rem - CHUNK == 0 else rem - 4
        else:
            take = rem
        out.append((c0, take))
        c0 += take
        rem -= take
    return out


_GROUPS = [(g0, min(TGROUP, NLEFT - g0)) for g0 in range(0, NLEFT, TGROUP)]


def _build():
    nc = bacc.Bacc("TRN2", target_bir_lowering=False, debug=False,
                   num_devices=N_CORES)
    x_in = nc.dram_tensor("x", [BL, F * D], f16, kind="ExternalInput").ap()
    # xt[d, ((bt*F + f)*BT + r)] = x[bt*BT + r, f, d]
    xt_in = nc.dram_tensor("xt", [D, NBT * F * BT], f16,
                           kind="ExternalInput").ap()
    # wt[d, p*D + e] = W[p, e, d]
    wt_in = nc.dram_tensor("wt", [D, P * D], f16, kind="ExternalInput").ap()
    out = nc.dram_tensor("out", [BL, P * D], f16, kind="ExternalOutput").ap()

    mult = mybir.AluOpType.mult

    with tile.TileContext(nc) as tc:
        with (
            tc.tile_pool(name="consts", bufs=1) as consts,
            tc.tile_pool(name="xp", bufs=2) as xp,
            tc.tile_pool(name="xtp", bufs=2) as xtp,
            tc.tile_pool(name="otp", bufs=3) as otp,
            tc.tile_pool(name="pm16p", bufs=6) as pm16p,
            tc.tile_pool(name="psm", bufs=8, space="PSUM") as psm,
        ):
            # one weight tile per field group -> matmuls of group g only
            # depend on weight load g
            wt_g = []
            for gi, (g0, gn) in enumerate(_GROUPS):
                c0 = _off(g0) * D
                c1 = _off(g0 + gn) * D
                t = consts.tile([D, c1 - c0], f16, tag=f"wt{gi}")
                wt_g.append(t)

            route_idx = 0
            for bt in range(NBT):
                rows = slice(bt * BT, (bt + 1) * BT)
                x_tile = xp.tile([BT, F * D], f16, tag="x")
                nc.sync.dma_start(out=x_tile, in_=x_in[rows, :])
                xt_tile = xtp.tile([D, F * BT], f16, tag="xt")
                nc.sync.dma_start(
                    out=xt_tile, in_=xt_in[:, bt * F * BT:(bt + 1) * F * BT])
                if bt == 0:
                    for gi, (g0, gn) in enumerate(_GROUPS):
                        c0 = _off(g0) * D
                        c1 = _off(g0 + gn) * D
                        nc.sync.dma_start(out=wt_g[gi], in_=wt_in[:, c0:c1])

                for gi, (g0, gn) in enumerate(_GROUPS):
                    gbase = _off(g0) * D
                    gsz = (_off(g0 + gn) - _off(g0)) * D
                    ot = otp.tile([BT, gsz], f16, tag="ot")
                    for i in range(g0, g0 + gn):
                        npair = F - 1 - i  # pairs (i, i+1..31), consecutive
                        p0 = _off(i)
                        lhsT = xt_tile[:, i * BT:(i + 1) * BT]  # [64, 128]
                        for c0, cn in _chunks(npair):
                            n = cn * D
                            ws = (p0 + c0) * D - gbase
                            pm = psm.tile([BT, n], f32, tag="mm")
                            nc.tensor.matmul(pm, lhsT,
                                             wt_g[gi][:, ws:ws + n],
                                             start=True, stop=True)
                            xj = x_tile[:, (i + 1 + c0) * D:
                                        (i + 1 + c0 + cn) * D]
                            ot_sl = ot[:, ws:ws + n]
                            route = _ROUTE_PATTERN[route_idx
                                                   % len(_ROUTE_PATTERN)]
                            route_idx += 1
                            if route == "D":
                                nc.vector.tensor_mul(ot_sl, pm, xj)
                            else:
                                pm16 = pm16p.tile([BT, n], f16, tag="pm16")
                                nc.scalar.copy(pm16, pm)
                                if route == "A":
                                    nc.vector.scalar_tensor_tensor(
                                        ot_sl, pm16, 1.0, xj, mult, mult)
                                else:
                                    nc.gpsimd.tensor_mul(ot_sl, pm16, xj)
                    nc.sync.dma_start(out=out[rows, gbase:gbase + gsz],
                                      in_=ot)
    nc.compile()
    return nc


def _get_nc():
    global _nc_cache
    if _nc_cache is None:
        _nc_cache = _build()
    return _nc_cache


def _prep_inputs(x, W):
    x16 = np.asarray(x, dtype=np.float16).reshape(N_CORES, BL, F * D)
    # per-core pre-transposed layout: [D, NBT, F, BT] flattened
    xt = np.ascontiguousarray(
        x16.reshape(N_CORES, NBT, BT, F, D).transpose(0, 4, 1, 3, 2)
    ).reshape(N_CORES, D, NBT * F * BT)
    wt = np.ascontiguousarray(
        np.asarray(W, dtype=np.float32).transpose(2, 0, 1)
    ).reshape(D, P * D).astype(np.float16)
    x16 = np.ascontiguousarray(x16)
    return x16, xt, wt


def _run(x, W, trace=False, trace_kwargs=None):
    x16, xt, wt = _prep_inputs(x, W)
    in_maps = [{"x": x16[c], "xt": xt[c], "wt": wt}
               for c in range(N_CORES)]
    res = run_bass_kernel_spmd(_get_nc(), in_maps, list(range(N_CORES)),
                               trace=trace, **(trace_kwargs or {}))
    outs = [res.results[c]["out"].reshape(BL, P, D) for c in range(N_CORES)]
    return np.concatenate(outs, axis=0).astype(np.float32), res


def kernel(x, W):
    out, _ = _run(x, W)
    return out


# revision 5
# speedup vs baseline: 1.4547x; 1.0819x over previous
"""Trainium2 Bass kernel for nn_BiLinearInteractionLayer.

Math: x:(B=4096, F=32, D=64) f32, W:(P=496, D=64, D=64) f32 (torch Linear
layout: out_e = sum_d in_d * W[e, d]).  For each pair p=(i,j), i<j:
    out[b, p, e] = (sum_d x[b,i,d] * W[p,e,d]) * x[b,j,e]

Strategy (data-parallel over batch, 8 cores x 512 rows):

The harness gate is rel_err < 2e-2 (normalized by the global max), so the
whole pipeline runs in fp16: single-pass k=64 fp16 matmuls (fp32 PSUM
accumulate), fp16 elementwise multiply, fp16 output stores that the host
widens back to f32 (measured ~8e-4 rel err).  Versus the fp32-exact
baseline this halves PE matmul passes, removes the hi/lo split entirely,
and halves HBM store traffic (the dominant cost: output is 65MB/core in
f32, 32.5MB in fp16).

Host preformatting: x is shipped twice in fp16 - natively (BL, F*D) for
the elementwise right-field operand, and pre-transposed per batch-tile
(64, bt*F*128) for the matmul stationary operand - plus W transposed to
wt[d, p*64+e] in fp16.  No on-chip transposes: the PE runs nothing but
the pair matmuls.

Per 128-row batch tile, per left field i: the npair (<= 31) pair matmuls
go in bank-aligned chunks of <= 8 pairs into ONE 4-bank PSUM tile
(npair*64 f32 <= 7936B), then a SINGLE per-field evict / elementwise
multiply amortizes the fixed per-instruction costs (ACT evict has ~260ns
of access latency + decode; v2 paid that 304x, this pays it <= 124x).
The combine is routed per field across three paths, balanced at build
time with measured per-element rates:
  D: DVE tensor_mul direct from PSUM (1x: fp32 operand)
  A: ACT evicts PSUM->SBUF fp16, DVE tensor_mul all-fp16/SBUF (2x_1p)
  P: ACT evicts, GPSIMD does the multiply (GPSIMD has no PSUM port)
Outputs accumulate in per-group (4 left fields) fp16 tiles and store once
per group: 8 stores/bt with 4-15KB contiguous runs per partition.

DMA queue split: loads issue on the Activation HWDGE ring, stores on the
Sync ring.  v2 put both on Sync, so the blocking semaphore wait ahead of
batch-tile N's stores also held up batch-tile N+1's loads (in-order
queue) and broke the inter-tile pipeline.

Consecutive matmul chunks of a field share the stationary operand, so
chunks 2..4 set InstMatmult.ldweights=False and ride the LDWEIGHTS of
chunk 1 (measured 128ns apiece, 304 -> 124 loads).

HBM traffic/core: 32.5MB out + 4MB wt + 4MB x (2 layouts) = 40.5MB.
"""
import numpy as np

import concourse.bacc as bacc
import concourse.tile as tile
import concourse.mybir as mybir
from concourse.bass_utils import run_bass_kernel_spmd

B = 4096
F = 32
D = 64
P = F * (F - 1) // 2  # 496
N_CORES = 8
BL = B // N_CORES     # 512 rows per core
BT = 128              # batch tile (SBUF partitions)
NBT = BL // BT        # 4 batch tiles per core
CHUNK = 8             # pairs per matmul chunk (8*64 = 512 = one PSUM bank)
TGROUP = 4            # left fields per output-store group
NLEFT = F - 1         # left fields 0..30

f32 = mybir.dt.float32
f16 = mybir.dt.float16

_nc_cache = None


def _off(i):
    """Pair index of the first pair with left field i."""
    return 31 * i - i * (i - 1) // 2


def _chunks(npair):
    # bank-aligned: chunk c starts at pair offset 8*c so every chunk's
    # f32 PSUM region stays inside one 2KB bank
    return [(c0, min(CHUNK, npair - c0)) for c0 in range(0, npair, CHUNK)]


_GROUPS = [(g0, min(TGROUP, NLEFT - g0)) for g0 in range(0, NLEFT, TGROUP)]

# measured per-instruction engine costs (ns) from the v2 trace / cost model:
# rate ns/elem (per-partition-lane), fixed ns/instruction
_ACT_RATE, _ACT_FIX = 0.833, 263.0
_DVE1_RATE, _DVE1_FIX = 1.042, 107.0   # tensor_tensor with PSUM f32 operand
_DVE2_RATE, _DVE2_FIX = 0.521, 75.0    # tensor_tensor all-SBUF fp16 (2x_1p)
_POOL_RATE, _POOL_FIX = 2.78, 330.0    # gpsimd tensor_tensor
_DMA_ISSUE_ACT = 667.0                 # HWDGE issue cost on the ACT queue


class _Balancer:
    """Greedy per-field route chooser minimizing the max engine load."""

    def __init__(self):
        self.act = 0.0
        self.dve = 0.0
        self.pool = 0.0

    def pick(self, e):
        # candidate route -> (act+, dve+, pool+)
        cand = {
            "D": (0.0, _DVE1_RATE * e + _DVE1_FIX, 0.0),
            "A": (_ACT_RATE * e + _ACT_FIX, _DVE2_RATE * e + _DVE2_FIX, 0.0),
            "P": (_ACT_RATE * e + _ACT_FIX, 0.0, _POOL_RATE * e + _POOL_FIX),
        }
        best, best_load = None, None
        for r, (a, d, p) in cand.items():
            load = max(self.act + a, self.dve + d, self.pool + p)
            if best_load is None or load < best_load:
                best, best_load = r, load
        a, d, p = cand[best]
        self.act += a
        self.dve += d
        self.pool += p
        return best


def _build():
    nc = bacc.Bacc("TRN2", target_bir_lowering=False, debug=False,
                   num_devices=N_CORES)
    x_in = nc.dram_tensor("x", [BL, F * D], f16, kind="ExternalInput").ap()
    # xt[d, ((bt*F + f)*BT + r)] = x[bt*BT + r, f, d]
    xt_in = nc.dram_tensor("xt", [D, NBT * F * BT], f16,
                           kind="ExternalInput").ap()
    # wt[d, p*D + e] = W[p, e, d]
    wt_in = nc.dram_tensor("wt", [D, P * D], f16, kind="ExternalInput").ap()
    out = nc.dram_tensor("out", [BL, P * D], f16, kind="ExternalOutput").ap()

    bal = _Balancer()

    with tile.TileContext(nc) as tc:
        with (
            tc.tile_pool(name="consts", bufs=1) as consts,
            tc.tile_pool(name="xp", bufs=2) as xp,
            tc.tile_pool(name="xtp", bufs=2) as xtp,
            tc.tile_pool(name="otp", bufs=3) as otp,
            tc.tile_pool(name="pm16p", bufs=4) as pm16p,
            tc.tile_pool(name="psm", bufs=2, space="PSUM") as psm,
        ):
            # one weight tile per field group -> matmuls of group g only
            # depend on weight load g
            wt_g = []
            for gi, (g0, gn) in enumerate(_GROUPS):
                c0 = _off(g0) * D
                c1 = _off(g0 + gn) * D
                t = consts.tile([D, c1 - c0], f16, tag=f"wt{gi}")
                wt_g.append(t)

            for bt in range(NBT):
                rows = slice(bt * BT, (bt + 1) * BT)
                x_tile = xp.tile([BT, F * D], f16, tag="x")
                nc.scalar.dma_start(out=x_tile, in_=x_in[rows, :])
                xt_tile = xtp.tile([D, F * BT], f16, tag="xt")
                nc.scalar.dma_start(
                    out=xt_tile, in_=xt_in[:, bt * F * BT:(bt + 1) * F * BT])
                bal.act += 2 * _DMA_ISSUE_ACT
                if bt == 0:
                    for gi, (g0, gn) in enumerate(_GROUPS):
                        c0 = _off(g0) * D
                        c1 = _off(g0 + gn) * D
                        nc.scalar.dma_start(out=wt_g[gi], in_=wt_in[:, c0:c1])
                        bal.act += _DMA_ISSUE_ACT

                for gi, (g0, gn) in enumerate(_GROUPS):
                    gbase = _off(g0) * D
                    gsz = (_off(g0 + gn) - _off(g0)) * D
                    ot = otp.tile([BT, gsz], f16, tag="ot")
                    for i in range(g0, g0 + gn):
                        npair = F - 1 - i  # pairs (i, i+1..31), consecutive
                        p0 = _off(i)
                        n_i = npair * D
                        lhsT = xt_tile[:, i * BT:(i + 1) * BT]  # [64, 128]
                        # one 4-bank PSUM tile holds the whole field
                        pm = psm.tile([BT, 4 * CHUNK * D], f32, tag="mm")
                        for ci, (c0, cn) in enumerate(_chunks(npair)):
                            n = cn * D
                            ws = (p0 + c0) * D - gbase
                            mm = nc.tensor.matmul(
                                pm[:, c0 * D:c0 * D + n], lhsT,
                                wt_g[gi][:, ws:ws + n],
                                start=True, stop=True)
                            if ci > 0:
                                # stationary unchanged: skip the reload
                                mm.ins.ldweights = False
                        xj = x_tile[:, (i + 1) * D:(i + 1) * D + n_i]
                        ot_sl = ot[:, (p0 * D - gbase):(p0 * D - gbase) + n_i]
                        route = bal.pick(n_i)
                        if route == "D":
                            nc.vector.tensor_mul(ot_sl, pm[:, :n_i], xj)
                        else:
                            pm16 = pm16p.tile([BT, n_i], f16, tag="pm16")
                            nc.scalar.copy(pm16, pm[:, :n_i])
                            if route == "A":
                                nc.vector.tensor_mul(ot_sl, pm16, xj)
                            else:
                                nc.gpsimd.tensor_mul(ot_sl, pm16, xj)
                    nc.sync.dma_start(out=out[rows, gbase:gbase + gsz],
                                      in_=ot)
    nc.compile()
    return nc


def _get_nc():
    global _nc_cache
    if _nc_cache is None:
        _nc_cache = _build()
    return _nc_cache


def _prep_inputs(x, W):
    x16 = np.asarray(x, dtype=np.float16).reshape(N_CORES, BL, F * D)
    # per-core pre-transposed layout: [D, NBT, F, BT] flattened
    xt = np.ascontiguousarray(
        x16.reshape(N_CORES, NBT, BT, F, D).transpose(0, 4, 1, 3, 2)
    ).reshape(N_CORES, D, NBT * F * BT)
    wt = np.ascontiguousarray(
        np.asarray(W, dtype=np.float32).transpose(2, 0, 1)
    ).reshape(D, P * D).astype(np.float16)
    x16 = np.ascontiguousarray(x16)
    return x16, xt, wt


def _run(x, W, trace=False, trace_kwargs=None):
    x16, xt, wt = _prep_inputs(x, W)
    in_maps = [{"x": x16[c], "xt": xt[c], "wt": wt}
               for c in range(N_CORES)]
    res = run_bass_kernel_spmd(_get_nc(), in_maps, list(range(N_CORES)),
                               trace=trace, **(trace_kwargs or {}))
    outs = [res.results[c]["out"].reshape(BL, P, D) for c in range(N_CORES)]
    return np.concatenate(outs, axis=0).astype(np.float32), res


def kernel(x, W):
    out, _ = _run(x, W)
    return out
